# revision 1
# baseline (speedup 1.0000x reference)
"""Contrastive loss (supervised NT-Xent style) on 8 Trainium2 NeuronCores.

Reference computation (N=8192, D=256, C=64 classes, T=0.5):
    sim   = (E @ E.T) / T
    max_i = row max of sim           (== sim_ii because rows are unit-norm)
    den_i = sum_{j != i} exp(sim_ij - max_i)
    loss  = mean over positive pairs (label match, i != j) of
            (log den_i + max_i - sim_ij)

Key algebraic restructuring: the positive-pair sim sum only enters the loss
globally, and
    sum_{i != j, lab_i == lab_j} sim_ij = (sum_c ||G_c||^2 - sum_i ||e_i||^2)/T
with G_c = sum of embeddings in class c.  So no per-pair masking is needed on
device; each core produces
    - den_full_i  (exp row sums, diagonal included -> host subtracts 1)
    - sumsq_i     (||e_i||^2, gives max_i = 2*sumsq_i)
    - g_part[c,d] (class sums over the core's 1024 rows)
and the host combines them with label bincounts into the scalar loss.

Sharding: rows split across 8 cores; each core computes its [1024, 8192] sim
block against the full embedding set (bf16 matmul, fp32 PSUM), with the exp
row-sum fused into the ScalarEngine activation pass via accum_out.
"""

import numpy as np
import ml_dtypes

import concourse.bass as bass
import concourse.bacc as bacc
import concourse.mybir as mybir
import concourse.tile as tile
from concourse.bass_utils import run_bass_kernel_spmd

N = 8192
D = 256
C = 64
TEMP = 0.5
N_CORES = 8
M = N // N_CORES          # 1024 rows per core
P = 128                   # partitions
MT = M // P               # 8 m-tiles per core
CHUNK = 512               # fp32 moving-operand / PSUM-bank width
QW = 2048                 # psum ping-pong tile width (4 banks)
NQ = N // QW              # 4 quarters per m-tile row

_F32 = mybir.dt.float32
_BF16 = mybir.dt.bfloat16
_BF16_NP = ml_dtypes.bfloat16


def build_nc(enable_asserts: bool = False):
    nc = bacc.Bacc(
        "TRN2",
        target_bir_lowering=False,
        debug=False,
        enable_asserts=enable_asserts,
        num_devices=N_CORES,
    )

    # chunk-major layout: [k, s, p, c] so each [128, 512] chunk is contiguous
    embT = nc.dram_tensor("embT", [2, N // CHUNK, P, CHUNK], _BF16, kind="ExternalInput").ap()
    embT_rows = nc.dram_tensor("embT_rows", [D, M], _BF16, kind="ExternalInput").ap()
    emb_rows = nc.dram_tensor("emb_rows", [M, D], _BF16, kind="ExternalInput").ap()
    onehot_rows = nc.dram_tensor("onehot_rows", [M, C], _BF16, kind="ExternalInput").ap()

    # row_stats[:, 0:8]  = den_full per m-tile,  row_stats[:, 8:16] = sumsq
    row_stats_d = nc.dram_tensor("row_stats", [P, 2 * MT], _F32, kind="ExternalOutput").ap()
    g_part_d = nc.dram_tensor("g_part", [C, D], _F32, kind="ExternalOutput").ap()

    with tile.TileContext(nc) as tc:
        with (
            tc.tile_pool(name="big", bufs=1) as big,
            tc.tile_pool(name="small", bufs=1) as small,
            tc.tile_pool(name="psum", bufs=2, space=bass.MemorySpace.PSUM) as psum,
        ):
            # ---- persistent SBUF residents ----
            embT_sb = [big.tile([P, N], _BF16, tag=f"embT{k}", name=f"embT_sb{k}") for k in range(2)]
            embTr_sb = [big.tile([P, M], _BF16, tag=f"embTr{k}", name=f"embTr_sb{k}") for k in range(2)]
            embr_sb = big.tile([P, MT * D], _BF16, tag="embr")      # natural rows
            oh_sb = big.tile([P, MT * C], _BF16, tag="oh")          # onehot rows

            # cols 0:32 = per-(m,q) partials; cols 32:34 = first-half partials
            # of the split (q0, m<2) tiles, folded in before the final reduce
            denom_parts = small.tile([P, MT * NQ + 2], _F32, tag="dparts")
            row_stats = small.tile([P, 2 * MT], _F32, tag="rstats")
            negmax = small.tile([P, MT], _F32, tag="negmax")
            sq_junk = small.tile([P, D], _F32, tag="sqjunk")
            g_sb = small.tile([C, D], _F32, tag="gsb")
            dummy = small.tile([P, 1], _F32, tag="dummy")
            warm = small.tile([P, P], _BF16, tag="warm")

            # ---- t=0: hoist the ACT exp table load; warm the PE HAM ----
            nc.gpsimd.memset(dummy[:], 0.0)
            nc.scalar.activation(
                out=dummy[:], in_=dummy[:],
                func=mybir.ActivationFunctionType.Exp, bias=0.0, scale=1.0,
            )
            nc.gpsimd.memset(warm[:], 0.0)
            warm_ps = psum.tile([P, P], _F32, tag="ps", name="warm_ps")
            for _ in range(24):
                nc.tensor.matmul(warm_ps[:], lhsT=warm[:], rhs=warm[:], start=True, stop=True)

            # ---- input DMAs (issue order == priority order) ----
            # lhsT first (first matmuls need it), then embT in consumption
            # order; emb_rows m0 early for the first negmax. q2/q3 stream on
            # the gpsimd SWDGE queue in parallel with the sync HWDGE queue.
            # Split the pre-first-EXP stream across both HWDGE queues:
            # sync: lhsT + k0 of q0/q1;  scalar (idle until first EXP): k1 of
            # q0/q1.  Everything later goes on sync.
            nc.sync.dma_start(out=embTr_sb[0][:], in_=embT_rows[0:P, :])
            nc.scalar.dma_start(out=embTr_sb[1][:], in_=embT_rows[P:2 * P, :])
            for s in range(QW // CHUNK):
                nc.sync.dma_start(
                    out=embT_sb[0][:, s * CHUNK:(s + 1) * CHUNK],
                    in_=embT[0, s],
                )
                nc.scalar.dma_start(
                    out=embT_sb[1][:, s * CHUNK:(s + 1) * CHUNK],
                    in_=embT[1, s],
                )
            nc.sync.dma_start(out=embr_sb[:, 0:D], in_=emb_rows[0:P, :])
            nc.scalar.dma_start(
                out=embT_sb[1][:, QW:2 * QW].rearrange("p (s c) -> p s c", c=CHUNK),
                in_=embT[1, QW // CHUNK:2 * QW // CHUNK].rearrange("s p c -> p s c"),
            )
            nc.sync.dma_start(
                out=embr_sb[:, D:].rearrange("p (m d) -> p m d", d=D),
                in_=emb_rows[P:, :].rearrange("(m p) d -> p m d", p=P),
            )
            nc.sync.dma_start(
                out=embT_sb[0][:, QW:2 * QW].rearrange("p (s c) -> p s c", c=CHUNK),
                in_=embT[0, QW // CHUNK:2 * QW // CHUNK].rearrange("s p c -> p s c"),
            )
            for q in range(2, NQ):
                for k in range(2):
                    nc.sync.dma_start(
                        out=embT_sb[k][:, q * QW:(q + 1) * QW].rearrange("p (s c) -> p s c", c=CHUNK),
                        in_=embT[k, q * QW // CHUNK:(q + 1) * QW // CHUNK].rearrange("s p c -> p s c"),
                    )
            nc.sync.dma_start(
                out=oh_sb[:].rearrange("p (m c) -> p m c", c=C),
                in_=onehot_rows[:].rearrange("(m p) c -> p m c", p=P),
            )

            # ---- per-row sumsq (-> max_i = 2*sumsq_i) ----
            # (tensor_tensor_reduce crashes TRN2 here; use mul + reduce)
            for m in range(MT):
                nc.vector.tensor_mul(
                    sq_junk[:],
                    embr_sb[:, m * D:(m + 1) * D],
                    embr_sb[:, m * D:(m + 1) * D],
                )
                nc.vector.tensor_reduce(
                    out=row_stats[:, MT + m:MT + m + 1],
                    in_=sq_junk[:],
                    axis=mybir.AxisListType.X,
                    op=mybir.AluOpType.add,
                )
                # per-m so the first ACT op doesn't wait on all 8 sumsq
                nc.vector.tensor_scalar_mul(
                    out=negmax[:, m:m + 1],
                    in0=row_stats[:, MT + m:MT + m + 1],
                    scalar1=-2.0,
                )

            # ---- main loop: sim chunks + fused exp row-sum ----
            # q outer / m inner: all 8 m-tiles consume quarter q while the
            # DMA stream for quarters q+1.. runs behind the compute.
            for q in range(NQ):
                for m in range(MT):
                    ps = psum.tile([P, QW], _F32, tag="ps")
                    for k in range(2):
                        for c4 in range(QW // CHUNK):
                            col = q * QW + c4 * CHUNK
                            nc.tensor.matmul(
                                ps[:, c4 * CHUNK:(c4 + 1) * CHUNK],
                                lhsT=embTr_sb[k][:, m * P:(m + 1) * P],
                                rhs=embT_sb[k][:, col:col + CHUNK],
                                start=(k == 0),
                                stop=(k == 1),
                            )
                    nc.scalar.activation(
                        out=ps[:],
                        in_=ps[:],
                        func=mybir.ActivationFunctionType.Exp,
                        bias=negmax[:, m:m + 1],
                        scale=2.0,
                        accum_out=denom_parts[:, m * NQ + q:m * NQ + q + 1],
                    )

            # ---- class sums over this core's rows: g[c, d] ----
            # (after the main loop: lowest priority, fills PE idle slack)
            g_ps = psum.tile([C, D], _F32, tag="ps")
            for j in range(MT):
                nc.tensor.matmul(
                    g_ps[:],
                    lhsT=oh_sb[:, j * C:(j + 1) * C],
                    rhs=embr_sb[:, j * D:(j + 1) * D],
                    start=(j == 0),
                    stop=(j == MT - 1),
                )
            nc.vector.tensor_copy(g_sb[:], g_ps[:])
            nc.sync.dma_start(out=g_part_d[:], in_=g_sb[:])

            # ---- fold quarter partials -> den_full per m-tile ----
            # fold the split-tile first-half partials into the q0 slots
            for m in range(2):
                nc.vector.tensor_add(
                    denom_parts[:, m * NQ:m * NQ + 1],
                    denom_parts[:, m * NQ:m * NQ + 1],
                    denom_parts[:, MT * NQ + m:MT * NQ + m + 1],
                )
            # per-m so only the last reduce waits on the final EXP
            for m in range(MT):
                nc.vector.tensor_reduce(
                    out=row_stats[:, m:m + 1],
                    in_=denom_parts[:, m * NQ:(m + 1) * NQ],
                    axis=mybir.AxisListType.X,
                    op=mybir.AluOpType.add,
                )
            nc.sync.dma_start(out=row_stats_d[:], in_=row_stats[:])

    nc.compile()
    return nc


_NC_CACHE = None


def _get_nc():
    global _NC_CACHE
    if _NC_CACHE is None:
        _NC_CACHE = build_nc()
    return _NC_CACHE


def make_in_maps(embeddings: np.ndarray, labels: np.ndarray):
    emb = np.asarray(embeddings, dtype=np.float32)
    labels = np.asarray(labels).astype(np.int64)
    emb16 = emb.astype(_BF16_NP)
    embT16 = np.ascontiguousarray(emb16.T)
    # chunk-major: [k, s, p, c] with each [128, 512] chunk contiguous
    embT_t = np.ascontiguousarray(
        embT16.reshape(2, P, N // CHUNK, CHUNK).transpose(0, 2, 1, 3)
    )
    onehot = (labels[:, None] == np.arange(C)[None, :]).astype(_BF16_NP)

    in_maps = []
    for c in range(N_CORES):
        r0, r1 = c * M, (c + 1) * M
        in_maps.append(
            {
                "embT": embT_t,
                "embT_rows": np.ascontiguousarray(embT16[:, r0:r1]),
                "emb_rows": np.ascontiguousarray(emb16[r0:r1, :]),
                "onehot_rows": np.ascontiguousarray(onehot[r0:r1, :]),
            }
        )
    return in_maps


def finalize(results, labels: np.ndarray) -> np.float32:
    labels = np.asarray(labels).astype(np.int64)
    den_full = np.empty(N, dtype=np.float64)
    sumsq = np.empty(N, dtype=np.float64)
    G = np.zeros((C, D), dtype=np.float64)
    for c in range(N_CORES):
        rs = np.asarray(results[c]["row_stats"], dtype=np.float64)  # [P, 2*MT]
        for m in range(MT):
            base = c * M + m * P
            den_full[base:base + P] = rs[:, m]
            sumsq[base:base + P] = rs[:, MT + m]
        G += np.asarray(results[c]["g_part"], dtype=np.float64)

    counts = np.bincount(labels, minlength=C)
    npos = counts[labels] - 1.0
    n_pos = npos.sum()

    max_i = 2.0 * sumsq
    den = den_full - 1.0            # drop the diagonal exp(0) term
    logden = np.log(den)
    pos_sim_total = 2.0 * ((G * G).sum() - sumsq.sum())  # (1/T) * (...)
    numer = (npos * (logden + max_i)).sum() - pos_sim_total
    return np.float32(numer / n_pos)


def _run(inputs, trace: bool = False, **kwargs):
    nc = _get_nc()
    in_maps = make_in_maps(inputs["embeddings"], inputs["epitope_labels"])
    return run_bass_kernel_spmd(nc, in_maps, list(range(N_CORES)), trace=trace, **kwargs)


def kernel(embeddings, epitope_labels) -> np.ndarray:
    res = _run({"embeddings": embeddings, "epitope_labels": epitope_labels})
    return finalize(res.results, epitope_labels)



# revision 10
# speedup vs baseline: 1.1699x; 1.1699x over previous
"""Contrastive loss (supervised NT-Xent style) on 8 Trainium2 NeuronCores.

Reference (N=8192, D=256, C=64 classes, T=0.5):
    sim   = (E @ E.T) / T
    loss  = mean over positive pairs (label match, i != j) of
            (log den_i + max_i - sim_ij),   den_i = sum_{j != i} exp(sim_ij - max_i)

Because rows are unit-norm, the row max is the diagonal sim_ii = 2, and the
loss is invariant to the (detached) shift, so a CONSTANT shift of -2 is used
everywhere - no per-row max needed.

Key structural trick: sim is symmetric, so only the upper triangle of the
16x16 grid of [512,512] blocks needs the expensive exp pass.  Each core c
computes two row panels over cyclic column bands (inputs are pre-rotated by
512*c on the host so the program is SPMD-identical):
    panel A: rows [0:512)    x cols [0:4608)     (9 chunks; chunk 0 = diagonal)
    panel B: rows [4096:4608) x cols [4096:8192) (8 chunks; chunk 0 = diagonal)
Every unordered block pair {R,C} is covered exactly once.  A block's exp
contributes row sums to den[rows] and - via a replicated-ones matmul on the
PE (column sums) - to den[cols]; the host scatter-adds both into den[8192].

The positive-pair sim sum enters globally:
    sum_{i!=j, lab eq} sim_ij = 2*(sum_c ||G_c||^2 - N)
with G_c the class sums, computed on-device in bf16 (g_part per core).

Per core: fp8(e4m3, x8-scaled) DoubleRow matmuls (K=256 in one instruction)
-> PSUM fp32 -> ACT exp (scale 2/64, bias -2) -> SBUF bf16, DVE row-sum
reduces, PE ones-matmul column sums -> DVE copy -> DMA out.
"""

import numpy as np
import ml_dtypes

import concourse.bass as bass
import concourse.bacc as bacc
import concourse.mybir as mybir
import concourse.tile as tile
from concourse.bass_utils import run_bass_kernel_spmd

N = 8192
D = 256
C = 64
N_CORES = 8
P = 128
NB = 16                   # 512-wide blocks
BK = 512                  # block/chunk width
SCALE = 8.0               # fp8 input scale; psum = 64 * (e_i . e_j)
ACT_SCALE = 2.0 / (SCALE * SCALE)   # exp(2*dot - 2)

# groups: (panel, qa) - qa is the rotated chunk index of the rhs columns.
# panel A: lhsT rows from chunk 0, rhs chunks 0..8 (chunk 0 = diagonal)
# panel B: lhsT rows from chunk 8, rhs chunks 8..15 (chunk 8 = diagonal)
GROUPS = [(0, q) for q in range(9)] + [(8, q) for q in range(8, 16)]
NG = len(GROUPS)          # 17
NCS = sum(1 for (q0, qa) in GROUPS if qa != q0)  # 15 column-sum chunks

_F32 = mybir.dt.float32
_BF16 = mybir.dt.bfloat16
_FP8 = mybir.dt.float8e4
_BF16_NP = ml_dtypes.bfloat16
_FP8_NP = ml_dtypes.float8_e4m3


def build_nc(enable_asserts: bool = False):
    nc = bacc.Bacc(
        "TRN2",
        target_bir_lowering=False,
        debug=False,
        enable_asserts=enable_asserts,
        num_devices=N_CORES,
    )

    # DoubleRow chunk-major layout: embT8[q, p, k*512 + c] = emb_rot[q*512+c, k*128+p] * 8
    embT8_d = nc.dram_tensor("embT8", [NB, P, 2 * BK], _FP8, kind="ExternalInput").ap()
    embN_d = nc.dram_tensor("embN", [P, 8 * D], _BF16, kind="ExternalInput").ap()
    oh_d = nc.dram_tensor("onehot", [P, 8 * C], _BF16, kind="ExternalInput").ap()

    den_d = nc.dram_tensor("den_rows", [P, 8], _F32, kind="ExternalOutput").ap()
    colsum_d = nc.dram_tensor("colsum", [1, NCS * BK], _F32, kind="ExternalOutput").ap()
    g_d = nc.dram_tensor("g_part", [C, D], _F32, kind="ExternalOutput").ap()

    with tile.TileContext(nc) as tc:
        with (
            tc.tile_pool(name="big", bufs=1) as big,
            tc.tile_pool(name="small", bufs=1) as small,
            tc.tile_pool(name="psum", bufs=2, space=bass.MemorySpace.PSUM) as psum,
        ):
            embT8_sb = big.tile([P, NB * 2 * BK], _FP8, tag="embT8")
            exp_sb = big.tile([P, NG * 4 * BK], _BF16, tag="exp")
            embN_sb = big.tile([P, 8 * D], _BF16, tag="embN")
            oh_sb = big.tile([P, 8 * C], _BF16, tag="oh")

            colsum_acc = small.tile([P, NCS * BK], _F32, tag="csacc")
            rs_parts = small.tile([P, NG * 4], _BF16, tag="rsparts")
            den_sb = small.tile([P, 8], _F32, tag="den")
            g_sb = small.tile([C, D], _F32, tag="gsb")
            ones_sb = small.tile([P, P], _BF16, tag="ones")
            dummy = small.tile([P, 1], _F32, tag="dummy")
            negtwo = small.tile([P, 1], _F32, tag="negtwo")

            # ---- t=0: hoist the ACT exp table load; warm + p-state ramp the PE ----
            nc.gpsimd.memset(dummy[:], 0.0)
            nc.scalar.activation(
                out=dummy[:], in_=dummy[:],
                func=mybir.ActivationFunctionType.Exp, bias=0.0, scale=1.0,
            )
            nc.gpsimd.memset(ones_sb[:], 1.0)
            nc.gpsimd.memset(negtwo[:], -2.0)
            warm_ps = psum.tile([P, P], _F32, tag="ps", name="warm_ps")
            for _ in range(20):
                nc.tensor.matmul(warm_ps[:], lhsT=ones_sb[:], rhs=ones_sb[:], start=True, stop=True)

            # ---- input DMAs, issue order == consumption order ----
            # embT8 chunks on the sync HWDGE queue; first chunks alone for
            # fast ramp, the rest batched.  embN/onehot on the vector queue.
            for q0, cnt in ((0, 1), (1, 1), (2, 2), (4, 4), (8, 4), (12, 4)):
                if cnt == 1:
                    nc.sync.dma_start(
                        out=embT8_sb[:, q0 * 2 * BK:(q0 + 1) * 2 * BK],
                        in_=embT8_d[q0],
                    )
                else:
                    nc.sync.dma_start(
                        out=embT8_sb[:, q0 * 2 * BK:(q0 + cnt) * 2 * BK].rearrange(
                            "p (q x) -> p q x", x=2 * BK
                        ),
                        in_=embT8_d[q0:q0 + cnt].rearrange("q p x -> p q x"),
                    )
            nc.gpsimd.dma_start(out=embN_sb[:], in_=embN_d[:])
            nc.gpsimd.dma_start(out=oh_sb[:], in_=oh_d[:])

            # [p, q, k, c] view of the fp8 operand buffer
            embT8_v = embT8_sb[:].rearrange("p (q k c) -> p q k c", k=2, c=BK)

            # ---- main loop over the 17 block-column groups ----
            pend_cs = None        # (ps_tile, cs_idx) colsum awaiting its DVE copy
            cs_idx = 0
            with nc.allow_low_precision("bf16 row-sum partials; rel err << gate"):
                for gi, (q0, qa) in enumerate(GROUPS):
                    ps = psum.tile([P, 4 * BK], _F32, tag="ps")
                    lhs_v = embT8_sb[:, q0 * 2 * BK:(q0 + 1) * 2 * BK].rearrange(
                        "p (k c) -> p k c", k=2
                    )
                    rhs_v = embT8_sb[:, qa * 2 * BK:(qa + 1) * 2 * BK].rearrange(
                        "p (k c) -> p k c", k=2
                    )
                    for m in range(4):
                        nc.tensor.matmul(
                            ps[:, m * BK:(m + 1) * BK],
                            lhsT=lhs_v[:, :, m * P:(m + 1) * P],
                            rhs=rhs_v,
                            start=True,
                            stop=True,
                            perf_mode=mybir.MatmulPerfMode.DoubleRow,
                        )
                    # previous group's colsum matmuls go behind this group's
                    # sim matmuls on the PE so they never stall it.
                    if pend_cs is not None:
                        _emit_colsum(nc, ones_sb, pend_cs[0], exp_sb, pend_cs[2], colsum_acc, pend_cs[1])
                        pend_cs = None
                    exp_slice = exp_sb[:, gi * 4 * BK:(gi + 1) * 4 * BK]
                    nc.scalar.activation(
                        out=exp_slice,
                        in_=ps[:],
                        func=mybir.ActivationFunctionType.Exp,
                        bias=negtwo[:],
                        scale=ACT_SCALE,
                    )
                    nc.vector.tensor_reduce(
                        out=rs_parts[:, gi * 4:(gi + 1) * 4],
                        in_=exp_slice.rearrange("p (s c) -> p s c", c=BK),
                        axis=mybir.AxisListType.X,
                        op=mybir.AluOpType.add,
                    )
                    if qa != q0:
                        pend_cs = (ps, cs_idx, gi)
                        cs_idx += 1
                if pend_cs is not None:
                    _emit_colsum(nc, ones_sb, pend_cs[0], exp_sb, pend_cs[2], colsum_acc, pend_cs[1])
                    pend_cs = None

            # ---- class sums over this core's 1024 rows ----
            g_ps = psum.tile([C, D], _F32, tag="ps")
            for s in range(8):
                nc.tensor.matmul(
                    g_ps[:],
                    lhsT=oh_sb[:, s * C:(s + 1) * C],
                    rhs=embN_sb[:, s * D:(s + 1) * D],
                    start=(s == 0),
                    stop=(s == 7),
                )
            nc.vector.tensor_copy(g_sb[:], g_ps[:])
            nc.sync.dma_start(out=g_d[:], in_=g_sb[:])

            # ---- fold row-sum partials -> den per slab ----
            rsv = rs_parts[:].rearrange("p (g s) -> p s g", s=4)
            for s in range(4):
                nc.vector.tensor_reduce(
                    out=den_sb[:, s:s + 1],
                    in_=rsv[:, s:s + 1, 0:9],
                    axis=mybir.AxisListType.X,
                    op=mybir.AluOpType.add,
                )
                nc.vector.tensor_reduce(
                    out=den_sb[:, 4 + s:5 + s],
                    in_=rsv[:, s:s + 1, 9:17],
                    axis=mybir.AxisListType.X,
                    op=mybir.AluOpType.add,
                )
            nc.sync.dma_start(out=den_d[:], in_=den_sb[:])
            nc.sync.dma_start(out=colsum_d[:], in_=colsum_acc[0:1, :])

    nc.compile()
    return nc


def _emit_colsum(nc, ones_sb, ps, exp_sb, gi, colsum_acc, idx):
    """Column sums of group gi's exp block: replicated-ones matmul over the 4
    slabs accumulating into the group's (now ACT-drained) first PSUM bank,
    then a DVE copy out to SBUF."""
    for m in range(4):
        nc.tensor.matmul(
            ps[:, 0:BK],
            lhsT=ones_sb[:],
            rhs=exp_sb[:, (gi * 4 + m) * BK:(gi * 4 + m + 1) * BK],
            start=(m == 0),
            stop=(m == 3),
        )
    nc.vector.tensor_copy(colsum_acc[:, idx * BK:(idx + 1) * BK], ps[:, 0:BK])


_NC_CACHE = None


def _get_nc():
    global _NC_CACHE
    if _NC_CACHE is None:
        _NC_CACHE = build_nc()
    return _NC_CACHE


def make_in_maps(embeddings: np.ndarray, labels: np.ndarray):
    emb = np.asarray(embeddings, dtype=np.float32)
    labels = np.asarray(labels).astype(np.int64)
    emb8 = (emb * SCALE).astype(_FP8_NP)
    emb16 = emb.astype(_BF16_NP)
    onehot = (labels[:, None] == np.arange(C)[None, :]).astype(_BF16_NP)

    in_maps = []
    for c in range(N_CORES):
        rot = (np.arange(N) + BK * c) % N
        e8r = emb8[rot]                      # [N, D] rotated
        X = np.ascontiguousarray(e8r.T)      # [D, N]
        embT8 = np.ascontiguousarray(
            X.reshape(2, P, NB, BK).transpose(2, 1, 0, 3).reshape(NB, P, 2 * BK)
        )
        rows_idx = np.concatenate([rot[0:4 * P], rot[4096:4096 + 4 * P]])
        embN = np.ascontiguousarray(
            emb16[rows_idx].reshape(8, P, D).transpose(1, 0, 2).reshape(P, 8 * D)
        )
        oh = np.ascontiguousarray(
            onehot[rows_idx].reshape(8, P, C).transpose(1, 0, 2).reshape(P, 8 * C)
        )
        in_maps.append({"embT8": embT8, "embN": embN, "onehot": oh})
    return in_maps


def finalize(results, labels: np.ndarray) -> np.float32:
    labels = np.asarray(labels).astype(np.int64)
    den = np.zeros(N, dtype=np.float64)
    G = np.zeros((C, D), dtype=np.float64)
    for c in range(N_CORES):
        rot = (np.arange(N) + BK * c) % N
        rows_idx = np.concatenate([rot[0:4 * P], rot[4096:4096 + 4 * P]])
        dr = np.asarray(results[c]["den_rows"], dtype=np.float64)   # [P, 8]
        for s in range(8):
            den[rows_idx[s * P:(s + 1) * P]] += dr[:, s]
        cs = np.asarray(results[c]["colsum"], dtype=np.float64).reshape(NCS, BK)
        i = 0
        for (q0, qa) in GROUPS:
            if qa == q0:
                continue
            den[rot[qa * BK:(qa + 1) * BK]] += cs[i]
            i += 1
        G += np.asarray(results[c]["g_part"], dtype=np.float64)

    den -= 1.0                     # drop the diagonal exp(0) term
    counts = np.bincount(labels, minlength=C)
    npos = counts[labels] - 1.0
    n_pos = npos.sum()
    numer = (npos * (np.log(den) + 2.0)).sum() - 2.0 * ((G * G).sum() - float(N))
    return np.float32(numer / n_pos)


def _run(inputs, trace: bool = False, **kwargs):
    nc = _get_nc()
    in_maps = make_in_maps(inputs["embeddings"], inputs["epitope_labels"])
    return run_bass_kernel_spmd(nc, in_maps, list(range(N_CORES)), trace=trace, **kwargs)


def kernel(embeddings, epitope_labels) -> np.ndarray:
    res = _run({"embeddings": embeddings, "epitope_labels": epitope_labels})
    return finalize(res.results, epitope_labels)
